# revision 14
# baseline (speedup 1.0000x reference)
"""Trainium2 Bass kernel for the EdgeMask problem.

Computes, for h (B,T,N,d), I_full (B,T,N,N), MLP params W1 (2d,hid) b1 (hid,)
W2 (hid,) b2 (1,):
    li = h @ W1[:d]; lj = h @ W1[d:]
    hid = relu(li[:,:,:,None,:] + lj[:,:,None,:,:] + b1)
    M = sigmoid(hid @ W2 + b2);  I_sparse = I_full * M
Returns (I_sparse, M).

Sharding: data-parallel over B across 8 NeuronCores (B=8), no collectives.

Per-core layout (per (t) slice, N=128 nodes, d=128, K=32 hidden):
  - hT = h[t].T via PE transpose (d on partitions)
  - liT/ljT via col-tiled PE matmuls with W1a/W1b as stationary operands.
    Partition stacking p = 32*gp + k (4 replicas of the 32 hidden units).
    "Group" g covers rows i in {g, g+32, g+64, g+96} (i = g + 32*gp).
      R[32gp+k, j]  = lj[j,k] + b1[k]        (replicated 4x, ACT adds b1)
      S[32gp+k, g]  = li[g+32gp, k]          (li "stack", fp32)
  - Pointwise (the N^2*K hot loop): for each group g one fused op
      hid_g = max(R + S[:,g], 0)   -- DVE tensor_scalar(add,max) / ACT Relu+bias
  - Reduce over k via PE: lhsT = blockdiag(W2 x4) (128,4), col-tiled 4-way,
    rhs = 4 groups' hid packed into (128,512):
      logits'[i=4w+c+32m, j] at PSUM[32q+m, 128c+j]  (w = 4*half + q)
  - Compact 2 PSUM banks -> dense (128,128) via DMA, sigmoid(+b2) on ACT,
    I_full * M on DVE, DMA out.
"""

import functools

import numpy as np

import concourse.bass as bass
import concourse.mybir as mybir
import concourse.tile as tile
from concourse import bacc

F32 = mybir.dt.float32
F16 = mybir.dt.float16

B = 8
T = 32
N = 128
D = 128
K = 32  # hidden
NCORES = 8

AFT = mybir.ActivationFunctionType
ALU = mybir.AluOpType

# dtype of the hid (pointwise+reduce) path: F16 -> DVE 4x mode, F32 exact
HID_DT = F16
HID_NP = np.float16 if HID_DT == F16 else np.float32

# how many of the 8 hid buffers are written by ACT (rest on DVE)
ACT_BUFS = 2


def _build(t_slices: int = T):
    nc = bacc.Bacc(
        "TRN2", target_bir_lowering=False, debug=False, num_devices=NCORES
    )

    ht_d = nc.dram_tensor("ht", [t_slices, D, N], HID_DT, kind="ExternalInput")
    i_d = nc.dram_tensor("ifull", [t_slices, N, N], F32, kind="ExternalInput")
    w1a_d = nc.dram_tensor("w1a", [D, K], HID_DT, kind="ExternalInput")
    w1b_d = nc.dram_tensor("w1b", [D, K], HID_DT, kind="ExternalInput")
    b1t_d = nc.dram_tensor("b1t", [128, 1], F32, kind="ExternalInput")
    wd_d = nc.dram_tensor("wd", [128, 32], HID_DT, kind="ExternalInput")
    b2t_d = nc.dram_tensor("b2t", [128, 1], F32, kind="ExternalInput")

    isp_d = nc.dram_tensor("isp", [t_slices, N, N], F32, kind="ExternalOutput")
    m_d = nc.dram_tensor("m", [t_slices, N, N], F32, kind="ExternalOutput")

    with tile.TileContext(nc) as tc:
        with (
            tc.tile_pool(name="const", bufs=1) as cpool,
            tc.tile_pool(name="hin", bufs=4) as hpool,
            tc.tile_pool(name="hts", bufs=3) as htpool,
            tc.tile_pool(name="rs", bufs=3) as rspool,
            tc.tile_pool(name="hid", bufs=16) as hidpool,
            tc.tile_pool(name="io", bufs=4) as iopool,
            tc.tile_pool(name="outp", bufs=3) as opool,
            tc.tile_pool(name="psum", bufs=2, space="PSUM") as ppool,
        ):
            w1a_sb = cpool.tile([D, K], HID_DT)
            nc.sync.dma_start(w1a_sb[:], w1a_d[:])
            w1b_sb = cpool.tile([D, K], HID_DT)
            nc.sync.dma_start(w1b_sb[:], w1b_d[:])
            b1t_sb = cpool.tile([128, 1], F32)
            nc.sync.dma_start(b1t_sb[:], b1t_d[:])
            wd_sb = cpool.tile([128, 32], HID_DT)
            nc.sync.dma_start(wd_sb[:], wd_d[:])
            b2t_sb = cpool.tile([128, 1], F32)
            nc.sync.dma_start(b2t_sb[:], b2t_d[:])

            for t in range(t_slices):
                # ---- load hT[t] (host pre-transposed, fp16) ----
                ht_sb = htpool.tile([D, N], HID_DT, tag="hts")
                nc.sync.dma_start(ht_sb[:], ht_d[t, :, :])

                # ---- liT / ljT, col-tiled (4 concurrent 32-col groups) ----
                lilj_ps = ppool.tile([128, N + K], F32, tag="lilj")
                for gp in range(4):
                    # ljT replicated: out[32gp+k, j] = lj[j, k]
                    nc.tensor.matmul(
                        lilj_ps[32 * gp : 32 * gp + 32, 0:N],
                        w1b_sb[:],
                        ht_sb[:],
                        tile_position=(0, 32 * gp),
                    )
                for gp in range(4):
                    # li stack: out[32gp+k, g] = li[g+32gp, k]
                    nc.tensor.matmul(
                        lilj_ps[32 * gp : 32 * gp + 32, N : N + K],
                        w1a_sb[:],
                        ht_sb[:, 32 * gp : 32 * gp + 32],
                        tile_position=(0, 32 * gp),
                    )

                # R = ljT_rep + b1 (cast to HID_DT); S = li stack (fp32)
                r_sb = rspool.tile([128, N], HID_DT, tag="r")
                nc.scalar.activation(
                    r_sb[:], lilj_ps[:, 0:N], AFT.Identity, bias=b1t_sb[:, 0:1]
                )
                s_sb = rspool.tile([128, K], F32, tag="s")
                nc.vector.tensor_copy(s_sb[:], lilj_ps[:, N : N + K])

                # ---- pointwise: hid_g = relu(R + S[:, g]) ----
                hbufs = [
                    hidpool.tile([128, 4 * N], HID_DT, tag="hid", name=f"hb{w}")
                    for w in range(8)
                ]
                for g in range(K):
                    w, c = divmod(g, 4)
                    dst = hbufs[w][:, c * N : (c + 1) * N]
                    # whole hbufs on one engine each, so each reduce matmul
                    # waits on a single producer engine (walrus sync-wait limit)
                    if w < ACT_BUFS:
                        nc.scalar.activation(
                            dst, r_sb[:], AFT.Relu, bias=s_sb[:, g : g + 1]
                        )
                    else:
                        nc.vector.tensor_scalar(
                            dst, r_sb[:], s_sb[:, g : g + 1], 0.0, ALU.add, ALU.max
                        )

                # ---- reduce over k on PE (col-tiled, 2 waves of 4) ----
                l_ps = [
                    ppool.tile([128, 4 * N], F32, tag="l0", name="l0"),
                    ppool.tile([128, 4 * N], F32, tag="l1", name="l1"),
                ]
                for w in range(8):
                    half, q = divmod(w, 4)
                    nc.tensor.matmul(
                        l_ps[half][32 * q : 32 * q + 32, :],
                        wd_sb[:],
                        hbufs[w][:],
                        tile_position=(0, 32 * q),
                    )

                # ---- sigmoid directly on (sparse) PSUM: the PSUM exit ----
                # used rows are {32q+m : q,m<4} subset of [0,100)
                msp = [
                    opool.tile([128, 4 * N], F32, tag="msp0", name="msp0"),
                    opool.tile([128, 4 * N], F32, tag="msp1", name="msp1"),
                ]
                for half in range(2):
                    nc.scalar.activation(
                        msp[half][:],
                        l_ps[half][:],
                        AFT.Sigmoid,
                        bias=b2t_sb[:, 0:1],
                    )

                # ---- permute sparse M -> dense (128,128) via on-chip DMAs ----
                # msp[half][32q+m, 128c+j] = M[32m+16half+4q+c, j]
                m_sb = opool.tile([128, N], F32, tag="m")
                for half in range(2):
                    for q in range(4):
                        for m in range(4):
                            i0 = 32 * m + 16 * half + 4 * q
                            p = 32 * q + m
                            nc.sync.dma_start(
                                m_sb[i0 : i0 + 4, :], msp[half][p : p + 1, :]
                            )
                i_sb = iopool.tile([N, N], F32, tag="i")
                nc.sync.dma_start(i_sb[:], i_d[t, :, :])
                isp_sb = opool.tile([N, N], F32, tag="isp")
                nc.vector.tensor_tensor(isp_sb[:], i_sb[:], m_sb[:], ALU.mult)
                nc.sync.dma_start(m_d[t, :, :], m_sb[:])
                nc.sync.dma_start(isp_d[t, :, :], isp_sb[:])

    nc.compile()
    return nc


def make_aux_inputs(W1, b1, W2, b2):
    W1 = np.asarray(W1)
    w1a = np.ascontiguousarray(W1[:D]).astype(HID_NP)
    w1b = np.ascontiguousarray(W1[D:]).astype(HID_NP)
    b1t = np.ascontiguousarray(np.tile(np.asarray(b1, np.float32), 4).reshape(128, 1))
    # col m carries W2 at partition-block (m % 4): every PSUM output row of the
    # reduce matmul is then a valid (replicated) logits row
    wd = np.zeros((128, 32), HID_NP)
    for m in range(32):
        gp = m % 4
        wd[32 * gp : 32 * gp + 32, m] = np.asarray(W2)
    b2t = np.full((128, 1), np.asarray(b2, np.float32)[0], np.float32)
    return {
        "w1a": w1a,
        "w1b": w1b,
        "b1t": b1t,
        "wd": wd,
        "b2t": b2t,
    }


TRACE = False
LAST_RESULTS = None


@functools.lru_cache(maxsize=1)
def _built_nc():
    return _build(T)


def kernel(**inputs):
    from concourse.bass_utils import run_bass_kernel_spmd

    h = np.asarray(inputs["h"])
    ht = np.ascontiguousarray(np.swapaxes(h, -1, -2)).astype(HID_NP)
    ifull = np.ascontiguousarray(np.asarray(inputs["I_full"], np.float32))
    aux = make_aux_inputs(inputs["W1"], inputs["b1"], inputs["W2"], inputs["b2"])

    nc = _built_nc()
    in_maps = [{"ht": ht[c], "ifull": ifull[c], **aux} for c in range(NCORES)]
    res = run_bass_kernel_spmd(
        nc, in_maps, core_ids=list(range(NCORES)), trace=TRACE
    )
    global LAST_RESULTS
    LAST_RESULTS = res
    isp = np.stack([res.results[c]["isp"] for c in range(NCORES)])
    m = np.stack([res.results[c]["m"] for c in range(NCORES)])
    return isp, m
